# revision 9
# baseline (speedup 1.0000x reference)
"""Distributed Trainium2 (8 NeuronCores) attention-head kernel, key-sharded.

Problem: single attention head with projections.
  q = Q @ Wq.T + bq ; k = K @ Wk.T + bk ; v = V @ Wv.T + bv
  x = (q @ k.T) / sqrt(64) ; x = x*m - 1e9*(1-m) ; p = softmax(x)
  y = p @ v
Shapes: Q/K/V [2, 4096, 1024] f32, mask [2, 4096, 4096] int32 -> y [2, 4096, 64] f32.

Strategy (8 cores): shard KEYS 4-ways within each batch (core c -> batch c//4,
key rows (c%4)*1024..+1024).  Each core projects its 1024-key K/V slice (k/v
projection work is NOT duplicated, unlike query-sharding which replicates it
4x), projects all 4096 batch queries (q-proj is half the size of k+v), computes
partial attention p = exp(qk/8)*m over its key range, and returns the partial
numerator and denominator yT[65, 4096] = [v|1]^T @ p.  The host sums the 4
partials per batch and divides - algebraically identical to the reference's
masked softmax (no fully-masked rows exist).

Layouts ([partitions, free], contraction on partitions, all matmul N=512):
  head:   kT[64,1024] = sum_j WkT_j.T @ ktile_j   (PSUM, dm-chunk j)
          vT likewise; v_aug[128, 8*65] = [vT.T | 1] via PE transposes
  per qg (8 query-groups of 512):
          qT[64,512] = sum_j WqT_j.T @ qtile      (+bias on DVE)
          sT_kc[128,512] = kT_kc.T @ qT (8 kc) ; p[:, kc] = exp(sT/8) (Scalar)
          p *= mask (two [128,2048] TENSOR_TENSORs: DVE half, GpSimd half)
          yT[65,512] += v_aug_kc.T @ p_kc  (8 kc, PSUM accum)
          out[:, qg] <- yT  (DVE copy to SBUF + DMA on the sync ring)
The y-matmuls of qg run one iteration later (software pipeline) so the PE
stream never waits on the exp->mask chain; scores(i) and y(i-1) interleave.
The LAST qg's mask is applied additively on the PE instead (an identity-lhsT
matmul accumulates (m-1)*128 bf16 into the scores PSUM before the exp), which
removes the exp->TT->y serial chain from the kernel tail.
DMA issue order == consumption order (per-engine FIFO queues preserve it).
"""

import numpy as np
import ml_dtypes

import concourse.bass as bass
import concourse.mybir as mybir
import concourse.tile as tile
from concourse import bacc
import concourse.bass_utils as bass_utils
from concourse.bass_utils import run_bass_kernel_spmd
from concourse.masks import make_identity

B, S, DM, DK = 2, 4096, 1024, 64
N_CORES = 8
GROUP = 4              # cores per batch
SK = S // GROUP        # key rows per core (1024)
NDM = DM // 128        # dm chunks (8)
NQG = 8                # query groups
QG = S // NQG          # 512 queries per group
NKC = SK // 128        # key chunks per core (8)
LAST = NQG - 1

F32 = mybir.dt.float32
BF16 = mybir.dt.bfloat16
FP8 = mybir.dt.float8e4

_last_results = None


def _build():
    nc = bacc.Bacc(None, target_bir_lowering=False)

    # kt/vt: [128, j(8) x 1024 keys] bf16; qt: [128, qg(8) x j(8) x 512] bf16
    # mt: [128, qg(0..6) x kc(8) x 512] fp8 multiplicative
    # mta: [128, kc(8) x 512] bf16 additive (m-1)*128 for the last qg
    kt_e = nc.declare_dram_parameter("kt", [128, NDM * SK], BF16, isOutput=False)
    vt_e = nc.declare_dram_parameter("vt", [128, NDM * SK], BF16, isOutput=False)
    qt_e = nc.declare_dram_parameter("qt", [128, NQG * NDM * QG], BF16, isOutput=False)
    mt_e = nc.declare_dram_parameter("mt", [128, LAST * NKC * QG], FP8, isOutput=False)
    mta_e = nc.declare_dram_parameter("mta", [128, NKC * QG], BF16, isOutput=False)
    w_e = nc.declare_dram_parameter("wqkv", [128, 3 * NDM * DK], BF16, isOutput=False)
    b_e = nc.declare_dram_parameter("bqkv", [DK, 3], F32, isOutput=False)
    out_e = nc.declare_dram_parameter("out", [DK + 1, S], F32, isOutput=True)

    QW = NDM * QG   # qt cols per qg (4096)
    MW = NKC * QG   # mask cols per qg (4096)

    with tile.TileContext(nc) as tc:
        with (
            tc.tile_pool(name="const", bufs=1) as cpool,
            tc.tile_pool(name="kvin", bufs=8) as kvpool,
            tc.tile_pool(name="qin", bufs=3) as qpool,
            tc.tile_pool(name="min", bufs=3) as mpool,
            tc.tile_pool(name="qT", bufs=3) as qtpool,
            tc.tile_pool(name="p", bufs=3) as ppool,
            tc.tile_pool(name="psum_pr", bufs=2, space="PSUM") as pproj,
            tc.tile_pool(name="psum_s", bufs=2, space="PSUM") as psT,
            tc.tile_pool(name="psum_y", bufs=2, space="PSUM") as pyT,
        ):
            # ---- DMA loads (sync HWDGE ring), in consumption order ----
            w_sb = cpool.tile([128, 3 * NDM * DK], BF16, tag="w")
            nc.sync.dma_start(w_sb[:], w_e[:])
            b_sb = cpool.tile([DK, 3], F32, tag="b")
            nc.sync.dma_start(b_sb[:], b_e[:])

            qt_t, mq_t = {}, {}

            def load_q(qg, split=False):
                t = qpool.tile([128, QW], BF16, tag="qt", name=f"qt_{qg}")
                if split:
                    nc.sync.dma_start(t[:, :QW // 2], qt_e[:, qg * QW:qg * QW + QW // 2])
                    nc.sync.dma_start(t[:, QW // 2:], qt_e[:, qg * QW + QW // 2:(qg + 1) * QW])
                else:
                    nc.sync.dma_start(t[:], qt_e[:, qg * QW:(qg + 1) * QW])
                qt_t[qg] = t

            def load_m(qg):
                if qg == LAST:
                    mq_t[qg] = mpool.tile([128, MW], BF16, tag="mta", name="mta")
                    nc.sync.dma_start(mq_t[qg][:], mta_e[:])
                else:
                    mq_t[qg] = mpool.tile([128, MW], FP8, tag="mt", name=f"mq_{qg}")
                    nc.sync.dma_start(mq_t[qg][:], mt_e[:, qg * MW:(qg + 1) * MW])

            kt_t, vt_t = [], []
            for jp in range(4):
                t = kvpool.tile([128, 2 * SK], BF16, tag="kv", name=f"kt_{jp}")
                nc.sync.dma_start(t[:], kt_e[:, jp * 2 * SK:(jp + 1) * 2 * SK])
                kt_t.append(t)
            load_q(0, split=True)
            load_m(0)
            load_q(1)
            for jp in range(4):
                t = kvpool.tile([128, 2 * SK], BF16, tag="kv", name=f"vt_{jp}")
                nc.sync.dma_start(t[:], vt_e[:, jp * 2 * SK:(jp + 1) * 2 * SK])
                vt_t.append(t)
            load_m(1)
            for qg in range(2, NQG):
                load_q(qg)
                load_m(qg)

            ident_bf = cpool.tile([128, 128], BF16, tag="ident")
            make_identity(nc, ident_bf[:])

            def wsl(which, j):  # weight chunk slice in w_sb
                return w_sb[:, (which * NDM + j) * DK:(which * NDM + j + 1) * DK]

            def proj_kv(which, src, dst):
                ph = [pproj.tile([DK, QG], F32, tag="proj", name=f"pr{which}h{h}")
                      for h in range(2)]
                for j in range(NDM):
                    jp, jl = divmod(j, 2)
                    for h in range(2):
                        c0 = jl * SK + h * QG
                        nc.tensor.matmul(
                            ph[h][:], lhsT=wsl(which, j),
                            rhs=src[jp][:, c0:c0 + QG],
                            start=(j == 0), stop=(j == NDM - 1),
                        )
                for h in range(2):
                    nc.vector.tensor_scalar_add(
                        dst[:, h * QG:(h + 1) * QG], ph[h][:],
                        b_sb[:, which:which + 1],
                    )

            kT_sb = cpool.tile([DK, SK], BF16, tag="kT")
            vT_sb = cpool.tile([DK, SK], BF16, tag="vT")
            v_aug = cpool.tile([128, NKC * (DK + 1)], BF16, tag="vaug")
            nc.gpsimd.memset(v_aug[:], 1.0)

            qT, p_t, yT_t = {}, {}, {}

            def issue_qproj(qg):
                ps = pproj.tile([DK, QG], F32, tag="proj", name=f"qps_{qg}")
                for j in range(NDM):
                    nc.tensor.matmul(
                        ps[:], lhsT=wsl(0, j),
                        rhs=qt_t[qg][:, j * QG:(j + 1) * QG],
                        start=(j == 0), stop=(j == NDM - 1),
                    )
                t = qtpool.tile([DK, QG], BF16, tag="qT", name=f"qT_{qg}")
                nc.vector.tensor_scalar_add(t[:], ps[:], b_sb[:, 0:1])
                qT[qg] = t

            def scores_half(i, half):
                for pr in range(half * 2, half * 2 + 2):
                    sT = psT.tile([128, 2 * QG], F32, tag="sT", name=f"sT_{i}_{pr}")
                    for k in range(2):
                        kc = pr * 2 + k
                        dst = sT[:, k * QG:(k + 1) * QG]
                        if i == LAST:
                            # additive mask fused into the scores PSUM
                            nc.tensor.matmul(
                                dst, lhsT=kT_sb[:, kc * 128:(kc + 1) * 128],
                                rhs=qT[i][:], start=True, stop=False,
                            )
                            nc.tensor.matmul(
                                dst, lhsT=ident_bf[:],
                                rhs=mq_t[i][:, kc * QG:(kc + 1) * QG],
                                start=False, stop=True,
                            )
                        else:
                            nc.tensor.matmul(
                                dst, lhsT=kT_sb[:, kc * 128:(kc + 1) * 128],
                                rhs=qT[i][:], start=True, stop=True,
                            )
                    nc.scalar.activation(
                        p_t[i][:, pr * 2 * QG:(pr + 1) * 2 * QG], sT[:],
                        mybir.ActivationFunctionType.Exp, scale=0.125,
                    )

            def mask_half(i, half):
                # half 0 on DVE, half 1 on GpSimd; [128, 2048] each
                eng = nc.vector if half == 0 else nc.gpsimd
                c = slice(half * 4 * QG, (half + 1) * 4 * QG)
                eng.tensor_mul(p_t[i][:, c], p_t[i][:, c], mq_t[i][:, c])

            def y_half(i, half):
                for kc in range(half * 4, half * 4 + 4):
                    nc.tensor.matmul(
                        yT_t[i][:],
                        lhsT=v_aug[:, kc * (DK + 1):(kc + 1) * (DK + 1)],
                        rhs=p_t[i][:, kc * QG:(kc + 1) * QG],
                        start=(kc == 0), stop=(kc == NKC - 1),
                    )

            y_all = cpool.tile([DK + 1, S], F32, tag="y_all")

            def store(i):
                nc.vector.tensor_copy(y_all[:, i * QG:(i + 1) * QG], yT_t[i][:])
                nc.sync.dma_start(
                    out_e[:, i * QG:(i + 1) * QG],
                    y_all[:, i * QG:(i + 1) * QG],
                )

            def new_tiles(i):
                p_t[i] = ppool.tile([128, NKC * QG], BF16, tag="p", name=f"p_{i}")
                yT_t[i] = pyT.tile([DK + 1, QG], F32, tag="yT", name=f"yT_{i}")

            # ---- head ----
            proj_kv(1, kt_t, kT_sb)
            new_tiles(0)
            issue_qproj(0)
            scores_half(0, 0)
            scores_half(0, 1)
            issue_qproj(1)
            mask_half(0, 0)
            mask_half(0, 1)
            proj_kv(2, vt_t, vT_sb)
            for kc in range(NKC):
                pt = psT.tile([128, 2 * QG], BF16, tag="sT", name=f"vtr_{kc}")
                nc.tensor.transpose(
                    pt[:, :DK], vT_sb[:, kc * 128:(kc + 1) * 128],
                    ident_bf[:DK, :DK],
                )
                nc.vector.tensor_copy(
                    v_aug[:, kc * (DK + 1):kc * (DK + 1) + DK], pt[:, :DK]
                )

            # ---- main loop, software-pipelined: scores(i) + y(i-1) ----
            for i in range(1, NQG):
                new_tiles(i)
                if i + 1 < NQG:
                    issue_qproj(i + 1)
                scores_half(i, 0)
                y_half(i - 1, 0)
                if i != LAST:
                    mask_half(i, 0)
                scores_half(i, 1)
                y_half(i - 1, 1)
                store(i - 1)
                if i != LAST:
                    mask_half(i, 1)

            y_half(LAST, 0)
            y_half(LAST, 1)
            store(LAST)

    nc.finalize()
    return nc


def _pack(at, w):
    """[R, W] -> [128, (R//128)*W]: row p gets rows {p, 128+p, ...}."""
    r = at.shape[0]
    return np.ascontiguousarray(
        at.reshape(r // 128, 128, w).transpose(1, 0, 2).reshape(128, -1)
    )


def kernel(Q, K, V, mask, Wq, bq, Wk, bk, Wv, bv):
    global _last_results
    bf16 = ml_dtypes.bfloat16
    fp8 = ml_dtypes.float8_e4m3

    w_p = np.concatenate(
        [_pack(W.T.astype(bf16), DK) for W in (Wq, Wk, Wv)], axis=1
    )
    b_p = np.ascontiguousarray(
        np.stack([bq, bk, bv], axis=1).astype(np.float32)
    )

    # qt per batch: [128, qg, j, 512]
    qt_b = []
    for b in range(B):
        a = Q[b].T.astype(bf16)                     # [1024 dm, 4096 q]
        a = a.reshape(NDM, 128, NQG, QG)            # [j, p, qg, q']
        qt_b.append(np.ascontiguousarray(
            a.transpose(1, 2, 0, 3).reshape(128, -1)
        ))

    in_maps = []
    for c in range(N_CORES):
        b, r = divmod(c, GROUP)
        rows = slice(r * SK, (r + 1) * SK)
        # mask tile: [128 s', qg, kc, 512 q']; last qg additive (m-1)*128
        m = mask[b].T[rows, :].astype(np.float32)   # [1024 s, 4096 q]
        m = m.reshape(NKC, 128, NQG, QG)            # [kc, p, qg, q']
        m = np.ascontiguousarray(m.transpose(1, 2, 0, 3))  # [p, qg, kc, q']
        mta = ((m[:, LAST] - 1.0) * 128.0).reshape(128, -1).astype(bf16)
        in_maps.append({
            "kt": _pack(np.ascontiguousarray(K[b, rows, :].T).astype(bf16), SK),
            "vt": _pack(np.ascontiguousarray(V[b, rows, :].T).astype(bf16), SK),
            "qt": qt_b[b],
            "mt": m[:, :LAST].reshape(128, -1).astype(fp8),
            "mta": mta,
            "wqkv": w_p,
            "bqkv": b_p,
        })

    nc = _build()
    res = run_bass_kernel_spmd(nc, in_maps, core_ids=list(range(N_CORES)))
    _last_results = res

    out = np.empty((B, S, DK), dtype=np.float32)
    for b in range(B):
        acc = np.zeros((DK + 1, S), dtype=np.float32)
        for r in range(GROUP):
            acc += res.results[b * GROUP + r]["out"]
        out[b] = (acc[:DK, :] / acc[DK:DK + 1, :]).T
    return out
